# revision 14
# baseline (speedup 1.0000x reference)
"""MoE (8-expert top-2 SwiGLU + shared MLP) Trainium2 kernel, 8-core data-parallel.

Data-parallel over the 8192 tokens (1024/core, no collectives). Each core:
router (fp32 matmul + softmax + top-2 via top-8 sort), positions via
triangular-ones matmul cumsum, per-expert dispatch matrices from the routing
mask (X_e^T = x^T selected columns, built as a mask matmul), SwiGLU expert
GEMMs over CAP-padded token batches, shared MLP accumulated in SBUF, and a
final combine via indirect row gather of each token's two expert outputs.
The 2/3 (moe) and 1/3 (shared) output scales are folded into w2/ws2 on host.

DT selects the matmul dtype for the heavy GEMMs: float32r (full-rate fp32,
rel err ~2.5e-4) or bfloat16 (~3e-3, less DMA + faster weight loads).
Router always runs fp32 (top-2 decisions need it), combine always fp32.
"""

import os
import sys
import numpy as np

sys.path.insert(0, "/opt/trn_rl_repo")

import ml_dtypes  # noqa: E402
from concourse import bacc, mybir  # noqa: E402
from concourse.bass import IndirectOffsetOnAxis  # noqa: E402
from concourse.tile import TileContext  # noqa: E402
from concourse.bass_utils import run_bass_kernel_spmd  # noqa: E402

F32 = mybir.dt.float32
F32R = mybir.dt.float32r
I32 = mybir.dt.int32
BF16 = mybir.dt.bfloat16
AF = mybir.ActivationFunctionType
OP = mybir.AluOpType

DT_NAME = os.environ.get("KERNEL_DT", "fp16")
F16 = mybir.dt.float16
DT = {"f32r": F32R, "bf16": BF16, "fp16": F16}[DT_NAME]
NP_DT = {"f32r": np.float32, "bf16": ml_dtypes.bfloat16, "fp16": np.float16}[DT_NAME]

D = 1024
E = 8
HID = 2048
SH = 2048
NCORES = 8
T = 8192
TC = T // NCORES
NTT = TC // 128   # 8 token tiles / core
NDC = D // 128    # 8
NHC = HID // 128  # 16
CAP = 304         # per-core per-expert capacity (measured max 294)
BIG = 4096.0
DW = 512 if DT in (BF16, F16) else 256   # GEMM2 moving width
NDQ = D // DW

_PROGRAM = None


def _build_program():
    nc = bacc.Bacc()

    x_tok = nc.declare_dram_parameter("x_tok", [TC, D], DT, isOutput=False)
    x_tr = nc.declare_dram_parameter("x_tr", [D, TC], DT, isOutput=False)
    x_t32 = nc.declare_dram_parameter("x_t32", [D, TC], F32, isOutput=False)
    wr = nc.declare_dram_parameter("wr", [D, E], F32, isOutput=False)
    # packed weights (see kernel() for host-side layouts)
    w1p = nc.declare_dram_parameter("w1p", [E, 8, 128, NDC, 256], DT, isOutput=False)
    w3p = nc.declare_dram_parameter("w3p", [E, 8, 128, NDC, 256], DT, isOutput=False)
    w2p = nc.declare_dram_parameter("w2p", [E, NDQ, 2, 128, 8, DW], DT, isOutput=False)
    ws1p = nc.declare_dram_parameter("ws1p", [8, 128, NDC, 256], DT, isOutput=False)
    ws3p = nc.declare_dram_parameter("ws3p", [8, 128, NDC, 256], DT, isOutput=False)
    ws2p = nc.declare_dram_parameter("ws2p", [4, NDQ, 128, 4, DW], DT, isOutput=False)
    uts = nc.declare_dram_parameter("uts", [128, 128], F32, isOutput=False)
    ones = nc.declare_dram_parameter("ones", [128, 128], F32, isOutput=False)
    iota_cap = nc.declare_dram_parameter("iota_cap", [128, CAP], F32, isOutput=False)
    ecap = nc.declare_dram_parameter("ecap", [128, E], F32, isOutput=False)
    tokid = nc.declare_dram_parameter("tokid", [128, NTT], DT, isOutput=False)
    out = nc.declare_dram_parameter("out", [TC, D], F32, isOutput=True)

    ybufs = [nc.dram_tensor(f"ybuf{q}", [E * CAP, DW], F32) for q in range(NDQ)]
    xg_dram = [nc.dram_tensor(f"xg{i}", [CAP, D], DT) for i in range(2)]

    xtok_v = x_tok.rearrange("(tt p) d -> p tt d", p=128)
    xtr_v = x_tr.rearrange("(dc p) t -> p dc t", p=128)
    xt32_v = x_t32.rearrange("(dc p) t -> p dc t", p=128)
    wr_v = wr.rearrange("(dc p) e -> p dc e", p=128)

    with TileContext(nc) as tc:
        with (
            tc.tile_pool(name="const", bufs=1) as cpool,
            tc.tile_pool(name="route", bufs=1) as rpool,
            tc.tile_pool(name="big", bufs=1) as bpool,
            tc.tile_pool(name="wts", bufs=2) as wpool,
            tc.tile_pool(name="work", bufs=2) as kpool,
            tc.tile_pool(name="ps_small", bufs=1, space="PSUM") as ps_s,
            tc.tile_pool(name="ps_uv", bufs=1, space="PSUM") as ps_uv,
            tc.tile_pool(name="ps_y", bufs=3, space="PSUM") as ps_y,
            tc.tile_pool(name="ps_x", bufs=2, space="PSUM") as ps_x,
        ):
            # ---- resident constants -------------------------------------
            uts_t = cpool.tile([128, 128], F32, tag="uts")
            ones_t = cpool.tile([128, 128], F32, tag="ones")
            iotac_t = cpool.tile([128, CAP], F32, tag="iotac")
            ecap_t = cpool.tile([128, E], F32, tag="ecap")
            nc.sync.dma_start(out=uts_t[:], in_=uts[:])
            nc.sync.dma_start(out=ones_t[:], in_=ones[:])
            nc.sync.dma_start(out=iotac_t[:], in_=iota_cap[:])
            nc.sync.dma_start(out=ecap_t[:], in_=ecap[:])
            wr_t = cpool.tile([128, NDC, E], F32, tag="wr")
            nc.sync.dma_start(out=wr_t[:], in_=wr_v)
            tokid_t = cpool.tile([128, NTT], DT, tag="tokid")
            nc.sync.dma_start(out=tokid_t[:], in_=tokid[:])

            xtr_t = bpool.tile([128, NDC, TC], DT, tag="xbig")
            nc.sync.dma_start(out=xtr_t[:], in_=xtr_v)

            outacc = bpool.tile([128, NTT, D], F32, tag="outacc")

            mask_all = rpool.tile([128, NTT, E], F32, tag="mask")
            m1_all = rpool.tile([128, NTT, E], F32, tag="m1")
            t8_all = rpool.tile([128, NTT, 8], F32, tag="t8")
            q_all = rpool.tile([128, NTT, E], F32, tag="q")
            off_all = rpool.tile([128, NTT, 2], I32, tag="off")

            # ---- Router + softmax + top-2 (x^T chunk-streamed) ----------
            lgacc = rpool.tile([128, NTT, E], F32, tag="lgacc")
            for dcq in range(4):
                xq = kpool.tile([128, 2, TC], F32, tag="xq", bufs=2)
                nc.sync.dma_start(out=xq[:], in_=xt32_v[:, dcq * 2:(dcq + 1) * 2, :])
                for tt in range(NTT):
                    ps_l = ps_s.tile([128, E], F32, tag="small")
                    for dc2 in range(2):
                        nc.tensor.matmul(
                            ps_l[:],
                            xq[:, dc2, tt * 128:(tt + 1) * 128],
                            wr_t[:, dcq * 2 + dc2, :],
                            start=(dc2 == 0), stop=(dc2 == 1),
                        )
                    if dcq == 0:
                        nc.vector.tensor_copy(lgacc[:, tt, :], ps_l[:])
                    else:
                        nc.vector.tensor_add(lgacc[:, tt, :], lgacc[:, tt, :], ps_l[:])
            for tt in range(NTT):
                lg = lgacc[:, tt, :]
                negmx = rpool.tile([128, 1], F32, tag="negmx")
                nc.vector.reduce_max(negmx[:], lg[:], axis=mybir.AxisListType.X,
                                     negate=True)
                ex = rpool.tile([128, E], F32, tag="ex")
                sm = rpool.tile([128, 1], F32, tag="sm")
                nc.scalar.activation(ex[:], lg[:], AF.Exp, bias=negmx[:],
                                     scale=1.0, accum_out=sm[:])
                rcp = rpool.tile([128, 1], F32, tag="rcp")
                nc.vector.reciprocal(rcp[:], sm[:])
                probs = rpool.tile([128, E], F32, tag="probs")
                nc.vector.tensor_scalar_mul(probs[:], ex[:], rcp[:])
                nc.vector.max(t8_all[:, tt, :], probs[:])
                nc.vector.tensor_tensor(
                    out=m1_all[:, tt, :], in0=probs[:],
                    in1=t8_all[:, tt, 0:1].to_broadcast([128, E]),
                    op=OP.is_ge)
                nc.vector.tensor_tensor(
                    out=mask_all[:, tt, :], in0=probs[:],
                    in1=t8_all[:, tt, 1:2].to_broadcast([128, E]),
                    op=OP.is_ge)

            # ---- positions (cumsum over token tiles), gather slots ------
            for tt in range(NTT):
                ps_p = ps_s.tile([128, E], F32, tag="small")
                for tp in range(tt):
                    nc.tensor.matmul(ps_p[:], ones_t[:], mask_all[:, tp, :],
                                     start=(tp == 0), stop=False)
                nc.tensor.matmul(ps_p[:], uts_t[:], mask_all[:, tt, :],
                                 start=(tt == 0), stop=True)
                qt = rpool.tile([128, E], F32, tag="qt")
                nc.vector.tensor_scalar(qt[:], mask_all[:, tt, :],
                                        scalar1=-BIG, scalar2=BIG,
                                        op0=OP.mult, op1=OP.add)
                nc.vector.tensor_add(q_all[:, tt, :], qt[:], ps_p[:])
                sl = rpool.tile([128, E], F32, tag="sl")
                nc.vector.tensor_add(sl[:], ps_p[:], ecap_t[:])
                m2 = rpool.tile([128, E], F32, tag="m2")
                nc.vector.tensor_sub(m2[:], mask_all[:, tt, :], m1_all[:, tt, :])
                s1m = rpool.tile([128, E], F32, tag="s1m")
                nc.vector.tensor_mul(s1m[:], sl[:], m1_all[:, tt, :])
                s1f = rpool.tile([128, 1], F32, tag="s1f")
                nc.vector.reduce_sum(s1f[:], s1m[:], axis=mybir.AxisListType.X)
                nc.vector.tensor_copy(off_all[:, tt, 0:1], s1f[:])
                s2m = rpool.tile([128, E], F32, tag="s2m")
                nc.vector.tensor_mul(s2m[:], sl[:], m2[:])
                s2f = rpool.tile([128, 1], F32, tag="s2f")
                nc.vector.reduce_sum(s2f[:], s2m[:], axis=mybir.AxisListType.X)
                nc.vector.tensor_copy(off_all[:, tt, 1:2], s2f[:])

            # ---- Shared MLP in SH quarters ------------------------------
            for sq in range(4):
                gs_t = bpool.tile([128, 4, TC], DT, tag="g")
                for hq in range(2):
                    hqg = sq * 2 + hq
                    wq1 = wpool.tile([128, NDC, 256], DT, tag="w1q")
                    nc.sync.dma_start(out=wq1[:], in_=ws1p[hqg])
                    wq3 = wpool.tile([128, NDC, 256], DT, tag="w3q")
                    nc.sync.dma_start(out=wq3[:], in_=ws3p[hqg])
                    for ht in range(2):
                        hg = hq * 2 + ht
                        for ts in range(2):
                            psu = ps_uv.tile([128, 512], F32, tag="psu")
                            psv = ps_uv.tile([128, 512], F32, tag="psv")
                            for dc in range(NDC):
                                nc.tensor.matmul(
                                    psu[:],
                                    wq1[:, dc, ht * 128:(ht + 1) * 128],
                                    xtr_t[:, dc, ts * 512:(ts + 1) * 512],
                                    start=(dc == 0), stop=(dc == NDC - 1))
                            for dc in range(NDC):
                                nc.tensor.matmul(
                                    psv[:],
                                    wq3[:, dc, ht * 128:(ht + 1) * 128],
                                    xtr_t[:, dc, ts * 512:(ts + 1) * 512],
                                    start=(dc == 0), stop=(dc == NDC - 1))
                            su = kpool.tile([128, 512], F32, tag="su")
                            nc.scalar.activation(su[:], psu[:], AF.Silu)
                            nc.vector.tensor_mul(
                                gs_t[:, hg, ts * 512:(ts + 1) * 512],
                                su[:], psv[:])
                for dq in range(NDQ):
                    w2q = wpool.tile([128, 4, DW], DT, tag="w2q")
                    nc.sync.dma_start(out=w2q[:], in_=ws2p[sq, dq])
                    for tt in range(NTT):
                        psy = ps_y.tile([128, DW], F32, tag="psy")
                        for hc in range(4):
                            nc.tensor.matmul(
                                psy[:],
                                gs_t[:, hc, tt * 128:(tt + 1) * 128],
                                w2q[:, hc, :],
                                start=(hc == 0), stop=(hc == 3))
                        if sq == 0:
                            nc.scalar.copy(outacc[:, tt, dq * DW:(dq + 1) * DW],
                                           psy[:])
                        else:
                            nc.vector.tensor_add(
                                outacc[:, tt, dq * DW:(dq + 1) * DW],
                                outacc[:, tt, dq * DW:(dq + 1) * DW],
                                psy[:])

            out_v = out.rearrange("(tt p) d -> p tt d", p=128)

            # ---- Experts: two halves of 4; GEMM2 grouped by d-half ------
            EH = E // 2
            for half in range(2):
                g_all = bpool.tile([128, EH, NHC, CAP], DT, tag="g",
                                   name=f"g_all_{half}")
                for ei in range(EH):
                    e = half * EH + ei
                    s_all = kpool.tile([128, NTT, CAP], DT, tag="s_all", bufs=2)
                    for tt in range(NTT):
                        nc.vector.tensor_tensor(
                            out=s_all[:, tt, :],
                            in0=q_all[:, tt, e:e + 1].to_broadcast([128, CAP]),
                            in1=iotac_t[:],
                            op=OP.is_equal)
                    xg = xg_dram[e % 2]
                    for ct in range(3):
                        cw = 128 if ct < 2 else CAP - 256
                        psi = ps_x.tile([128, 1], F32, tag="psx")
                        for tt in range(NTT):
                            nc.tensor.matmul(
                                psi[:cw],
                                s_all[:, tt, ct * 128:ct * 128 + cw],
                                tokid_t[:, tt:tt + 1],
                                start=(tt == 0), stop=(tt == NTT - 1))
                        idxf = kpool.tile([128, 1], F32, tag="idxf", bufs=2)
                        nc.scalar.copy(idxf[:cw], psi[:cw])
                        idxi = kpool.tile([128, 1], I32, tag="idxi", bufs=2)
                        nc.vector.tensor_copy(idxi[:cw], idxf[:cw])
                        xg_sb = kpool.tile([128, D], DT, tag="xg", bufs=2)
                        nc.gpsimd.indirect_dma_start(
                            out=xg_sb[:cw], out_offset=None,
                            in_=x_tok[:, :],
                            in_offset=IndirectOffsetOnAxis(ap=idxi[:cw], axis=0))
                        nc.sync.dma_start(out=xg[ct * 128:ct * 128 + cw, :],
                                          in_=xg_sb[:cw])
                    xe_t = kpool.tile([128, NDC, CAP], DT, tag="xe", bufs=2)
                    nc.sync.dma_start_transpose(out=xe_t[:], in_=xg[:, :])

                    for hq in range(8):
                        wq1 = wpool.tile([128, NDC, 256], DT, tag="w1q")
                        nc.sync.dma_start(out=wq1[:], in_=w1p[e, hq])
                        wq3 = wpool.tile([128, NDC, 256], DT, tag="w3q")
                        nc.sync.dma_start(out=wq3[:], in_=w3p[e, hq])
                        for ht in range(2):
                            hg = hq * 2 + ht
                            psu = ps_uv.tile([128, CAP], F32, tag="psu")
                            psv = ps_uv.tile([128, CAP], F32, tag="psv")
                            for dc in range(NDC):
                                nc.tensor.matmul(
                                    psu[:], wq1[:, dc, ht * 128:(ht + 1) * 128],
                                    xe_t[:, dc, :],
                                    start=(dc == 0), stop=(dc == NDC - 1))
                            for dc in range(NDC):
                                nc.tensor.matmul(
                                    psv[:], wq3[:, dc, ht * 128:(ht + 1) * 128],
                                    xe_t[:, dc, :],
                                    start=(dc == 0), stop=(dc == NDC - 1))
                            su = kpool.tile([128, CAP], F32, tag="su")
                            nc.scalar.activation(su[:], psu[:], AF.Silu)
                            nc.vector.tensor_mul(g_all[:, ei, hg, :], su[:], psv[:])

                # GEMM2 for this half's 4 experts, d-half (dq) outer
                for dq in range(NDQ):
                    for ei in range(EH):
                        e = half * EH + ei
                        psy_l = [ps_y.tile([128, DW], F32, tag="psy",
                                           name=f"psy_{e}_{dq}_{i}")
                                 for i in range(3)]
                        for qh in range(2):
                            w2q = wpool.tile([128, 8, DW], DT, tag="w2q")
                            nc.sync.dma_start(out=w2q[:], in_=w2p[e, dq, qh])
                            for ct in range(3):
                                cw = 128 if ct < 2 else CAP - 256
                                for hc in range(8):
                                    nc.tensor.matmul(
                                        psy_l[ct][:cw],
                                        g_all[:, ei, qh * 8 + hc,
                                              ct * 128:ct * 128 + cw],
                                        w2q[:, hc, :],
                                        start=(qh == 0 and hc == 0),
                                        stop=(qh == 1 and hc == 7))
                        for ct in range(3):
                            cw = 128 if ct < 2 else CAP - 256
                            ysb = kpool.tile([128, DW], F32, tag="ysb")
                            nc.scalar.copy(ysb[:cw], psy_l[ct][:cw])
                            nc.sync.dma_start(
                                out=ybufs[dq][e * CAP + ct * 128:
                                              e * CAP + ct * 128 + cw, :],
                                in_=ysb[:cw])

                    # after the LAST half finishes a d-half, combine it
                    if half == 1:
                        for tt in range(NTT):
                            y1 = kpool.tile([128, DW], F32, tag="late", bufs=3)
                            nc.gpsimd.indirect_dma_start(
                                out=y1[:], out_offset=None,
                                in_=ybufs[dq][:, :],
                                in_offset=IndirectOffsetOnAxis(
                                    ap=off_all[:, tt, 0:1], axis=0))
                            y2 = kpool.tile([128, DW], F32, tag="late2", bufs=3)
                            nc.gpsimd.indirect_dma_start(
                                out=y2[:], out_offset=None,
                                in_=ybufs[dq][:, :],
                                in_offset=IndirectOffsetOnAxis(
                                    ap=off_all[:, tt, 1:2], axis=0))
                            fin = kpool.tile([128, DW], F32, tag="fin", bufs=3)
                            nc.vector.tensor_scalar_mul(
                                fin[:], y1[:], scalar1=t8_all[:, tt, 0:1])
                            nc.vector.tensor_scalar_mul(
                                y2[:], y2[:], scalar1=t8_all[:, tt, 1:2])
                            nc.vector.tensor_add(fin[:], fin[:], y2[:])
                            nc.vector.tensor_add(
                                fin[:], fin[:],
                                outacc[:, tt, dq * DW:(dq + 1) * DW])
                            nc.sync.dma_start(
                                out=out_v[:, tt, dq * DW:(dq + 1) * DW],
                                in_=fin[:])

    nc.finalize()
    return nc


def _get_program():
    global _PROGRAM
    if _PROGRAM is None:
        _PROGRAM = _build_program()
    return _PROGRAM


XPOSE_ORDER = int(os.environ.get("XPOSE_ORDER", "0"))


def _pack_w13(w):
    # [E, D, HID] -> [E, hq, p, dc, col]; row order must match what
    # dma_start_transpose produces for xe ((dc,p) vs (p,dc) interleave)
    if XPOSE_ORDER == 0:
        return np.ascontiguousarray(
            w.reshape(E, NDC, 128, 8, 256).transpose(0, 3, 2, 1, 4).astype(NP_DT))
    return np.ascontiguousarray(
        w.reshape(E, 128, NDC, 8, 256).transpose(0, 3, 1, 2, 4).astype(NP_DT))


def _pack_w2(w):
    # [E, HID, D] -> [E, dq, qh, p, hcl, col]
    return np.ascontiguousarray(
        w.reshape(E, 2, 8, 128, NDQ, DW).transpose(0, 4, 1, 3, 2, 5).astype(NP_DT))


def _pack_ws13(w):
    # [D, SH] -> [hqg, p, dc, col]
    return np.ascontiguousarray(
        w.reshape(NDC, 128, 8, 256).transpose(2, 1, 0, 3).astype(NP_DT))


def _pack_ws2(w):
    # [SH, D] -> [sq, dq, p, hcl, col]
    return np.ascontiguousarray(
        w.reshape(4, 4, 128, NDQ, DW).transpose(0, 3, 2, 1, 4).astype(NP_DT))


def kernel(x, w_router, w1, w3, w2, ws1, ws3, ws2):
    x = np.asarray(x, dtype=np.float32)
    w_router = np.ascontiguousarray(np.asarray(w_router, dtype=np.float32))
    w1 = np.asarray(w1, dtype=np.float32)
    w3 = np.asarray(w3, dtype=np.float32)
    w2 = np.asarray(w2, dtype=np.float32) * (2.0 / 3.0)
    ws1 = np.asarray(ws1, dtype=np.float32)
    ws3 = np.asarray(ws3, dtype=np.float32)
    ws2 = np.asarray(ws2, dtype=np.float32) * (1.0 / 3.0)

    orig_shape = x.shape
    xf = np.ascontiguousarray(x.reshape(T, D))

    idx = np.arange(128, dtype=np.float32)
    uts = (idx[:, None] < idx[None, :]).astype(np.float32)
    ones = np.ones((128, 128), dtype=np.float32)
    iota_cap = np.broadcast_to(np.arange(CAP, dtype=np.float32), (128, CAP)).copy()
    ecap = np.broadcast_to(np.arange(E, dtype=np.float32) * CAP, (128, E)).copy()
    tokid = (np.arange(128, dtype=np.float32)[:, None]
             + 128.0 * np.arange(NTT, dtype=np.float32)[None, :]).astype(NP_DT)

    w1p, w3p = _pack_w13(w1), _pack_w13(w3)
    w2p = _pack_w2(w2)
    ws1p, ws3p = _pack_ws13(ws1), _pack_ws13(ws3)
    ws2p = _pack_ws2(ws2)

    nc = _get_program()

    in_maps = []
    for c in range(NCORES):
        xc = np.ascontiguousarray(xf[c * TC:(c + 1) * TC])
        xct = np.ascontiguousarray(xc.T)
        in_maps.append({
            "x_tok": xc.astype(NP_DT), "x_tr": xct.astype(NP_DT), "x_t32": xct,
            "wr": w_router,
            "w1p": w1p, "w3p": w3p, "w2p": w2p,
            "ws1p": ws1p, "ws3p": ws3p, "ws2p": ws2p,
            "uts": uts, "ones": ones, "iota_cap": iota_cap, "ecap": ecap,
            "tokid": tokid,
        })

    res = run_bass_kernel_spmd(nc, in_maps, list(range(NCORES)))
    out = np.concatenate([res.results[c]["out"] for c in range(NCORES)], axis=0)
    return out.reshape(orig_shape).astype(np.float32)


# revision 15
# speedup vs baseline: 1.0510x; 1.0510x over previous
"""MoE (8-expert top-2 SwiGLU + shared MLP) Trainium2 kernel, 8-core data-parallel.

Data-parallel over the 8192 tokens (1024/core, no collectives). Each core:
router (fp32 matmul + softmax + top-2 via top-8 sort), positions via
triangular-ones matmul cumsum, per-expert dispatch matrices from the routing
mask (X_e^T = x^T selected columns, built as a mask matmul), SwiGLU expert
GEMMs over CAP-padded token batches, shared MLP accumulated in SBUF, and a
final combine via indirect row gather of each token's two expert outputs.
The 2/3 (moe) and 1/3 (shared) output scales are folded into w2/ws2 on host.

DT selects the matmul dtype for the heavy GEMMs: float32r (full-rate fp32,
rel err ~2.5e-4) or bfloat16 (~3e-3, less DMA + faster weight loads).
Router always runs fp32 (top-2 decisions need it), combine always fp32.
"""

import os
import sys
import numpy as np

sys.path.insert(0, "/opt/trn_rl_repo")

import ml_dtypes  # noqa: E402
from concourse import bacc, mybir  # noqa: E402
from concourse.bass import IndirectOffsetOnAxis  # noqa: E402
from concourse.tile import TileContext  # noqa: E402
from concourse.bass_utils import run_bass_kernel_spmd  # noqa: E402

F32 = mybir.dt.float32
F32R = mybir.dt.float32r
I32 = mybir.dt.int32
BF16 = mybir.dt.bfloat16
AF = mybir.ActivationFunctionType
OP = mybir.AluOpType

DT_NAME = os.environ.get("KERNEL_DT", "fp16")
F16 = mybir.dt.float16
DT = {"f32r": F32R, "bf16": BF16, "fp16": F16}[DT_NAME]
NP_DT = {"f32r": np.float32, "bf16": ml_dtypes.bfloat16, "fp16": np.float16}[DT_NAME]

D = 1024
E = 8
HID = 2048
SH = 2048
NCORES = 8
T = 8192
TC = T // NCORES
NTT = TC // 128   # 8 token tiles / core
NDC = D // 128    # 8
NHC = HID // 128  # 16
CAP = 304         # per-core per-expert capacity (measured max 294)
BIG = 4096.0
DW = 512 if DT in (BF16, F16) else 256   # GEMM2 moving width
NDQ = D // DW

_PROGRAM = None


def _build_program():
    nc = bacc.Bacc()

    x_tok = nc.declare_dram_parameter("x_tok", [TC, D], DT, isOutput=False)
    x_tr = nc.declare_dram_parameter("x_tr", [D, TC], DT, isOutput=False)
    x_t32 = nc.declare_dram_parameter("x_t32", [D, TC], F32, isOutput=False)
    wr = nc.declare_dram_parameter("wr", [D, E], F32, isOutput=False)
    # packed weights (see kernel() for host-side layouts)
    w1p = nc.declare_dram_parameter("w1p", [E, 8, 128, NDC, 256], DT, isOutput=False)
    w3p = nc.declare_dram_parameter("w3p", [E, 8, 128, NDC, 256], DT, isOutput=False)
    w2p = nc.declare_dram_parameter("w2p", [E, NDQ, 2, 128, 8, DW], DT, isOutput=False)
    ws1p = nc.declare_dram_parameter("ws1p", [8, 128, NDC, 256], DT, isOutput=False)
    ws3p = nc.declare_dram_parameter("ws3p", [8, 128, NDC, 256], DT, isOutput=False)
    ws2p = nc.declare_dram_parameter("ws2p", [4, NDQ, 128, 4, DW], DT, isOutput=False)
    uts = nc.declare_dram_parameter("uts", [128, 128], F32, isOutput=False)
    ones = nc.declare_dram_parameter("ones", [128, 128], F32, isOutput=False)
    iota_cap = nc.declare_dram_parameter("iota_cap", [128, CAP], F32, isOutput=False)
    ecap = nc.declare_dram_parameter("ecap", [128, E], F32, isOutput=False)
    out = nc.declare_dram_parameter("out", [TC, D], F32, isOutput=True)

    ybufs = [nc.dram_tensor(f"ybuf{q}", [E * CAP, DW], F32) for q in range(NDQ)]

    xtok_v = x_tok.rearrange("(tt p) d -> p tt d", p=128)
    xtr_v = x_tr.rearrange("(dc p) t -> p dc t", p=128)
    xt32_v = x_t32.rearrange("(dc p) t -> p dc t", p=128)
    wr_v = wr.rearrange("(dc p) e -> p dc e", p=128)

    with TileContext(nc) as tc:
        with (
            tc.tile_pool(name="const", bufs=1) as cpool,
            tc.tile_pool(name="route", bufs=1) as rpool,
            tc.tile_pool(name="big", bufs=1) as bpool,
            tc.tile_pool(name="wts", bufs=2) as wpool,
            tc.tile_pool(name="work", bufs=2) as kpool,
            tc.tile_pool(name="ps_small", bufs=1, space="PSUM") as ps_s,
            tc.tile_pool(name="ps_uv", bufs=1, space="PSUM") as ps_uv,
            tc.tile_pool(name="ps_y", bufs=3, space="PSUM") as ps_y,
            tc.tile_pool(name="ps_x", bufs=2, space="PSUM") as ps_x,
        ):
            # ---- resident constants -------------------------------------
            uts_t = cpool.tile([128, 128], F32, tag="uts")
            ones_t = cpool.tile([128, 128], F32, tag="ones")
            iotac_t = cpool.tile([128, CAP], F32, tag="iotac")
            ecap_t = cpool.tile([128, E], F32, tag="ecap")
            nc.sync.dma_start(out=uts_t[:], in_=uts[:])
            nc.sync.dma_start(out=ones_t[:], in_=ones[:])
            nc.sync.dma_start(out=iotac_t[:], in_=iota_cap[:])
            nc.sync.dma_start(out=ecap_t[:], in_=ecap[:])
            wr_t = cpool.tile([128, NDC, E], F32, tag="wr")
            nc.sync.dma_start(out=wr_t[:], in_=wr_v)

            xtr_t = bpool.tile([128, NDC, TC], DT, tag="xbig")
            nc.sync.dma_start(out=xtr_t[:], in_=xtr_v)

            outacc = bpool.tile([128, NTT, D], F32, tag="outacc")

            mask_all = rpool.tile([128, NTT, E], F32, tag="mask")
            m1_all = rpool.tile([128, NTT, E], F32, tag="m1")
            t8_all = rpool.tile([128, NTT, 8], F32, tag="t8")
            q_all = rpool.tile([128, NTT, E], F32, tag="q")
            off_all = rpool.tile([128, NTT, 2], I32, tag="off")

            # ---- Router + softmax + top-2 (x^T chunk-streamed) ----------
            lgacc = rpool.tile([128, NTT, E], F32, tag="lgacc")
            for dcq in range(4):
                xq = kpool.tile([128, 2, TC], F32, tag="xq", bufs=2)
                nc.sync.dma_start(out=xq[:], in_=xt32_v[:, dcq * 2:(dcq + 1) * 2, :])
                for tt in range(NTT):
                    ps_l = ps_s.tile([128, E], F32, tag="small")
                    for dc2 in range(2):
                        nc.tensor.matmul(
                            ps_l[:],
                            xq[:, dc2, tt * 128:(tt + 1) * 128],
                            wr_t[:, dcq * 2 + dc2, :],
                            start=(dc2 == 0), stop=(dc2 == 1),
                        )
                    if dcq == 0:
                        nc.vector.tensor_copy(lgacc[:, tt, :], ps_l[:])
                    else:
                        nc.vector.tensor_add(lgacc[:, tt, :], lgacc[:, tt, :], ps_l[:])
            for tt in range(NTT):
                lg = lgacc[:, tt, :]
                negmx = rpool.tile([128, 1], F32, tag="negmx")
                nc.vector.reduce_max(negmx[:], lg[:], axis=mybir.AxisListType.X,
                                     negate=True)
                ex = rpool.tile([128, E], F32, tag="ex")
                sm = rpool.tile([128, 1], F32, tag="sm")
                nc.scalar.activation(ex[:], lg[:], AF.Exp, bias=negmx[:],
                                     scale=1.0, accum_out=sm[:])
                rcp = rpool.tile([128, 1], F32, tag="rcp")
                nc.vector.reciprocal(rcp[:], sm[:])
                probs = rpool.tile([128, E], F32, tag="probs")
                nc.vector.tensor_scalar_mul(probs[:], ex[:], rcp[:])
                nc.vector.max(t8_all[:, tt, :], probs[:])
                nc.vector.tensor_tensor(
                    out=m1_all[:, tt, :], in0=probs[:],
                    in1=t8_all[:, tt, 0:1].to_broadcast([128, E]),
                    op=OP.is_ge)
                nc.vector.tensor_tensor(
                    out=mask_all[:, tt, :], in0=probs[:],
                    in1=t8_all[:, tt, 1:2].to_broadcast([128, E]),
                    op=OP.is_ge)

            # ---- positions (cumsum over token tiles), gather slots ------
            for tt in range(NTT):
                ps_p = ps_s.tile([128, E], F32, tag="small")
                for tp in range(tt):
                    nc.tensor.matmul(ps_p[:], ones_t[:], mask_all[:, tp, :],
                                     start=(tp == 0), stop=False)
                nc.tensor.matmul(ps_p[:], uts_t[:], mask_all[:, tt, :],
                                 start=(tt == 0), stop=True)
                qt = rpool.tile([128, E], F32, tag="qt")
                nc.vector.tensor_scalar(qt[:], mask_all[:, tt, :],
                                        scalar1=-BIG, scalar2=BIG,
                                        op0=OP.mult, op1=OP.add)
                nc.vector.tensor_add(q_all[:, tt, :], qt[:], ps_p[:])
                sl = rpool.tile([128, E], F32, tag="sl")
                nc.vector.tensor_add(sl[:], ps_p[:], ecap_t[:])
                m2 = rpool.tile([128, E], F32, tag="m2")
                nc.vector.tensor_sub(m2[:], mask_all[:, tt, :], m1_all[:, tt, :])
                s1m = rpool.tile([128, E], F32, tag="s1m")
                nc.vector.tensor_mul(s1m[:], sl[:], m1_all[:, tt, :])
                s1f = rpool.tile([128, 1], F32, tag="s1f")
                nc.vector.reduce_sum(s1f[:], s1m[:], axis=mybir.AxisListType.X)
                nc.vector.tensor_copy(off_all[:, tt, 0:1], s1f[:])
                s2m = rpool.tile([128, E], F32, tag="s2m")
                nc.vector.tensor_mul(s2m[:], sl[:], m2[:])
                s2f = rpool.tile([128, 1], F32, tag="s2f")
                nc.vector.reduce_sum(s2f[:], s2m[:], axis=mybir.AxisListType.X)
                nc.vector.tensor_copy(off_all[:, tt, 1:2], s2f[:])

            # ---- Shared MLP in SH quarters ------------------------------
            for sq in range(4):
                gs_t = bpool.tile([128, 4, TC], DT, tag="g")
                for hq in range(2):
                    hqg = sq * 2 + hq
                    wq1 = wpool.tile([128, NDC, 256], DT, tag="w1q")
                    nc.sync.dma_start(out=wq1[:], in_=ws1p[hqg])
                    wq3 = wpool.tile([128, NDC, 256], DT, tag="w3q")
                    nc.sync.dma_start(out=wq3[:], in_=ws3p[hqg])
                    for ht in range(2):
                        hg = hq * 2 + ht
                        for ts in range(2):
                            psu = ps_uv.tile([128, 512], F32, tag="psu")
                            psv = ps_uv.tile([128, 512], F32, tag="psv")
                            for dc in range(NDC):
                                nc.tensor.matmul(
                                    psu[:],
                                    wq1[:, dc, ht * 128:(ht + 1) * 128],
                                    xtr_t[:, dc, ts * 512:(ts + 1) * 512],
                                    start=(dc == 0), stop=(dc == NDC - 1))
                            for dc in range(NDC):
                                nc.tensor.matmul(
                                    psv[:],
                                    wq3[:, dc, ht * 128:(ht + 1) * 128],
                                    xtr_t[:, dc, ts * 512:(ts + 1) * 512],
                                    start=(dc == 0), stop=(dc == NDC - 1))
                            su = kpool.tile([128, 512], F32, tag="su")
                            nc.scalar.activation(su[:], psu[:], AF.Silu)
                            nc.vector.tensor_mul(
                                gs_t[:, hg, ts * 512:(ts + 1) * 512],
                                su[:], psv[:])
                for dq in range(NDQ):
                    w2q = wpool.tile([128, 4, DW], DT, tag="w2q")
                    nc.sync.dma_start(out=w2q[:], in_=ws2p[sq, dq])
                    for tt in range(NTT):
                        psy = ps_y.tile([128, DW], F32, tag="psy")
                        for hc in range(4):
                            nc.tensor.matmul(
                                psy[:],
                                gs_t[:, hc, tt * 128:(tt + 1) * 128],
                                w2q[:, hc, :],
                                start=(hc == 0), stop=(hc == 3))
                        if sq == 0:
                            nc.scalar.copy(outacc[:, tt, dq * DW:(dq + 1) * DW],
                                           psy[:])
                        else:
                            nc.vector.tensor_add(
                                outacc[:, tt, dq * DW:(dq + 1) * DW],
                                outacc[:, tt, dq * DW:(dq + 1) * DW],
                                psy[:])

            # x token-major for dispatch (reuses xtr's slot)
            xtok_t = bpool.tile([128, NTT, D], DT, tag="xbig")
            nc.sync.dma_start(out=xtok_t[:], in_=xtok_v)
            out_v = out.rearrange("(tt p) d -> p tt d", p=128)

            # ---- Experts: two halves of 4; GEMM2 grouped by d-half ------
            EH = E // 2
            for half in range(2):
                g_all = bpool.tile([128, EH, NHC, CAP], DT, tag="g",
                                   name=f"g_all_{half}")
                for ei in range(EH):
                    e = half * EH + ei
                    s_all = kpool.tile([128, NTT, CAP], DT, tag="s_all", bufs=2)
                    for tt in range(NTT):
                        nc.vector.tensor_tensor(
                            out=s_all[:, tt, :],
                            in0=q_all[:, tt, e:e + 1].to_broadcast([128, CAP]),
                            in1=iotac_t[:],
                            op=OP.is_equal)
                    xe_t = kpool.tile([128, NDC, CAP], DT, tag="xe", bufs=2)
                    for dc in range(NDC):
                        psx = ps_x.tile([128, CAP], F32, tag="psx")
                        for tt in range(NTT):
                            nc.tensor.matmul(
                                psx[:],
                                xtok_t[:, tt, dc * 128:(dc + 1) * 128],
                                s_all[:, tt, :],
                                start=(tt == 0), stop=(tt == NTT - 1))
                        nc.scalar.copy(xe_t[:, dc, :], psx[:])

                    for hq in range(8):
                        wq1 = wpool.tile([128, NDC, 256], DT, tag="w1q")
                        nc.sync.dma_start(out=wq1[:], in_=w1p[e, hq])
                        wq3 = wpool.tile([128, NDC, 256], DT, tag="w3q")
                        nc.sync.dma_start(out=wq3[:], in_=w3p[e, hq])
                        for ht in range(2):
                            hg = hq * 2 + ht
                            psu = ps_uv.tile([128, CAP], F32, tag="psu")
                            psv = ps_uv.tile([128, CAP], F32, tag="psv")
                            for dc in range(NDC):
                                nc.tensor.matmul(
                                    psu[:], wq1[:, dc, ht * 128:(ht + 1) * 128],
                                    xe_t[:, dc, :],
                                    start=(dc == 0), stop=(dc == NDC - 1))
                            for dc in range(NDC):
                                nc.tensor.matmul(
                                    psv[:], wq3[:, dc, ht * 128:(ht + 1) * 128],
                                    xe_t[:, dc, :],
                                    start=(dc == 0), stop=(dc == NDC - 1))
                            su = kpool.tile([128, CAP], F32, tag="su")
                            nc.scalar.activation(su[:], psu[:], AF.Silu)
                            nc.vector.tensor_mul(g_all[:, ei, hg, :], su[:], psv[:])

                # GEMM2 for this half's 4 experts, d-half (dq) outer
                for dq in range(NDQ):
                    for ei in range(EH):
                        e = half * EH + ei
                        psy_l = [ps_y.tile([128, DW], F32, tag="psy",
                                           name=f"psy_{e}_{dq}_{i}")
                                 for i in range(3)]
                        for qh in range(2):
                            w2q = wpool.tile([128, 8, DW], DT, tag="w2q")
                            nc.sync.dma_start(out=w2q[:], in_=w2p[e, dq, qh])
                            for ct in range(3):
                                cw = 128 if ct < 2 else CAP - 256
                                for hc in range(8):
                                    nc.tensor.matmul(
                                        psy_l[ct][:cw],
                                        g_all[:, ei, qh * 8 + hc,
                                              ct * 128:ct * 128 + cw],
                                        w2q[:, hc, :],
                                        start=(qh == 0 and hc == 0),
                                        stop=(qh == 1 and hc == 7))
                        for ct in range(3):
                            cw = 128 if ct < 2 else CAP - 256
                            ysb = kpool.tile([128, DW], F32, tag="ysb")
                            nc.scalar.copy(ysb[:cw], psy_l[ct][:cw])
                            nc.sync.dma_start(
                                out=ybufs[dq][e * CAP + ct * 128:
                                              e * CAP + ct * 128 + cw, :],
                                in_=ysb[:cw])

                    # after the LAST half finishes a d-half, combine it
                    if half == 1:
                        for tt in range(NTT):
                            y1 = kpool.tile([128, DW], F32, tag="late", bufs=3)
                            nc.gpsimd.indirect_dma_start(
                                out=y1[:], out_offset=None,
                                in_=ybufs[dq][:, :],
                                in_offset=IndirectOffsetOnAxis(
                                    ap=off_all[:, tt, 0:1], axis=0))
                            y2 = kpool.tile([128, DW], F32, tag="late2", bufs=3)
                            nc.gpsimd.indirect_dma_start(
                                out=y2[:], out_offset=None,
                                in_=ybufs[dq][:, :],
                                in_offset=IndirectOffsetOnAxis(
                                    ap=off_all[:, tt, 1:2], axis=0))
                            fin = kpool.tile([128, DW], F32, tag="fin", bufs=3)
                            nc.vector.tensor_scalar_mul(
                                fin[:], y1[:], scalar1=t8_all[:, tt, 0:1])
                            nc.vector.tensor_scalar_mul(
                                y2[:], y2[:], scalar1=t8_all[:, tt, 1:2])
                            nc.vector.tensor_add(fin[:], fin[:], y2[:])
                            nc.vector.tensor_add(
                                fin[:], fin[:],
                                outacc[:, tt, dq * DW:(dq + 1) * DW])
                            nc.sync.dma_start(
                                out=out_v[:, tt, dq * DW:(dq + 1) * DW],
                                in_=fin[:])

    nc.finalize()
    return nc


def _get_program():
    global _PROGRAM
    if _PROGRAM is None:
        _PROGRAM = _build_program()
    return _PROGRAM


def _pack_w13(w):
    # [E, D, HID] -> [E, hq, p, dc, col] so each (e,hq) load is contiguous
    return np.ascontiguousarray(
        w.reshape(E, NDC, 128, 8, 256).transpose(0, 3, 2, 1, 4).astype(NP_DT))


def _pack_w2(w):
    # [E, HID, D] -> [E, dq, qh, p, hcl, col]
    return np.ascontiguousarray(
        w.reshape(E, 2, 8, 128, NDQ, DW).transpose(0, 4, 1, 3, 2, 5).astype(NP_DT))


def _pack_ws13(w):
    # [D, SH] -> [hqg, p, dc, col]
    return np.ascontiguousarray(
        w.reshape(NDC, 128, 8, 256).transpose(2, 1, 0, 3).astype(NP_DT))


def _pack_ws2(w):
    # [SH, D] -> [sq, dq, p, hcl, col]
    return np.ascontiguousarray(
        w.reshape(4, 4, 128, NDQ, DW).transpose(0, 3, 2, 1, 4).astype(NP_DT))


def kernel(x, w_router, w1, w3, w2, ws1, ws3, ws2):
    x = np.asarray(x, dtype=np.float32)
    w_router = np.ascontiguousarray(np.asarray(w_router, dtype=np.float32))
    w1 = np.asarray(w1, dtype=np.float32)
    w3 = np.asarray(w3, dtype=np.float32)
    w2 = np.asarray(w2, dtype=np.float32) * (2.0 / 3.0)
    ws1 = np.asarray(ws1, dtype=np.float32)
    ws3 = np.asarray(ws3, dtype=np.float32)
    ws2 = np.asarray(ws2, dtype=np.float32) * (1.0 / 3.0)

    orig_shape = x.shape
    xf = np.ascontiguousarray(x.reshape(T, D))

    idx = np.arange(128, dtype=np.float32)
    uts = (idx[:, None] < idx[None, :]).astype(np.float32)
    ones = np.ones((128, 128), dtype=np.float32)
    iota_cap = np.broadcast_to(np.arange(CAP, dtype=np.float32), (128, CAP)).copy()
    ecap = np.broadcast_to(np.arange(E, dtype=np.float32) * CAP, (128, E)).copy()

    w1p, w3p = _pack_w13(w1), _pack_w13(w3)
    w2p = _pack_w2(w2)
    ws1p, ws3p = _pack_ws13(ws1), _pack_ws13(ws3)
    ws2p = _pack_ws2(ws2)

    nc = _get_program()

    in_maps = []
    for c in range(NCORES):
        xc = np.ascontiguousarray(xf[c * TC:(c + 1) * TC])
        xct = np.ascontiguousarray(xc.T)
        in_maps.append({
            "x_tok": xc.astype(NP_DT), "x_tr": xct.astype(NP_DT), "x_t32": xct,
            "wr": w_router,
            "w1p": w1p, "w3p": w3p, "w2p": w2p,
            "ws1p": ws1p, "ws3p": ws3p, "ws2p": ws2p,
            "uts": uts, "ones": ones, "iota_cap": iota_cap, "ecap": ecap,
        })

    res = run_bass_kernel_spmd(nc, in_maps, list(range(NCORES)))
    out = np.concatenate([res.results[c]["out"] for c in range(NCORES)], axis=0)
    return out.reshape(orig_shape).astype(np.float32)
